# revision 15
# baseline (speedup 1.0000x reference)
"""AffineEdgeAttention Trainium2 kernel (fp16 I/O, PE-centric).

out[b, i, j] = head[b, i] . w_h + dep[b, j] . w_d + edge_b
with w_h = edge_W[0, :D], w_d = edge_W[0, D:].

Sharding: data-parallel over batch; 16 batches / 8 cores = 2 per core.

The correctness gate is max-abs/max-abs rel err < 2e-2, so all HBM
traffic runs in fp16 (inputs cast on host, output upcast on host):
per core 3+3 MiB loads + 4 MiB stores ~= 10.25 MiB -> ~29 us at the
358 GB/s per-NC HBM limit.

All the arithmetic lives on the otherwise-idle TensorEngine so no
vector engine paces the DMA stream:
  - head/dep load TRANSPOSED (fp16 xbar DMA): tiles [128 d, 1024 rows].
  - dot products: 12 accumulating K=128 matmuls per batch into one
    PSUM [2, 1024] tile. lhsT columns come from a [128, 14] w tile
    whose cols 0/13 are zero: dep k uses cols (7+k, 13) -> row0 += sd,
    row1 += 0; head k uses cols (0, 1+k) -> row0 += 0, row1 += sh.
  - rows land in a persistent SBUF [4, 1024] fp16 tile R = (sd+b, 1,
    1, sh) (ACT adds edge_b, memset ones rows once).
  - output chunk c: ONE K=2 matmul: lhsT=R[2:4, c*128:(c+1)*128]
    (ones, sh), rhs=R[0:2, :] (sdb, ones) -> out[i,j] = sdb[j]+sh[i].
  - V/ACT only evacuate PSUM -> SBUF fp16 (~0.7-1 us per chunk),
    stores split across the sync/scalar HWDGE rings.
edge_b is baked in at trace time via memset (it's known host-side).
"""

import sys

import numpy as np

for _p in ("/opt/trn_rl_repo", "/root/.axon_site/_ro/trn_rl_repo"):
    if _p not in sys.path:
        sys.path.insert(0, _p)

import concourse.bacc as bacc
import concourse.bass as bass
import concourse.tile as tile
from concourse import mybir
from concourse.bass_utils import run_bass_kernel_spmd

B, S, D = 16, 1024, 768
N_CORES = 8
BPC = B // N_CORES  # batches per core
P = 128
C = S // P  # 8 row-chunks of 128
KD = D // P  # 6 d-chunks of 128

F16 = mybir.dt.float16
F32 = mybir.dt.float32

# evacuation engine + store ring per output chunk (V->sync, A->scalar)
EVAC_ENG = ["V", "A", "V", "A", "V", "A", "V", "A"]


def build_program(b_const: float) -> bass.Bass:
    nc = bacc.Bacc("TRN2", target_bir_lowering=False, debug=False)
    head = nc.dram_tensor("head", [BPC, S, D], F16, kind="ExternalInput").ap()
    dep = nc.dram_tensor("dep", [BPC, S, D], F16, kind="ExternalInput").ap()
    w = nc.dram_tensor("edge_W", [1, 2 * D], F16, kind="ExternalInput").ap()
    out = nc.dram_tensor("out", [BPC, S, S], F16, kind="ExternalOutput").ap()

    out_v = out.rearrange("b (c p) j -> b c p j", p=P)

    with tile.TileContext(nc) as tc:
        with (
            tc.tile_pool(name="singles", bufs=1) as singles,
            tc.tile_pool(name="loads", bufs=2 * KD) as loads,
            tc.tile_pool(name="outs", bufs=6) as outs,
            tc.tile_pool(name="psrows", bufs=1, space="PSUM") as psrows,
            tc.tile_pool(name="psout", bufs=2, space="PSUM") as psout,
            tc.tile_pool(name="pswt", bufs=1, space="PSUM") as pswt,
        ):
            # ---- w column tile: [128, 14]: cols 0..5 = w_h chunks,
            # col 6 = 0, cols 7..12 = w_d chunks, col 13 = 0; built via
            # 12 tiny PE transpose matmuls ----
            w_row = singles.tile([1, 2 * D], F16)
            nc.scalar.dma_start(out=w_row, in_=w)
            one1 = singles.tile([1, 1], F16)
            nc.vector.memset(one1, 1.0)
            wcol_ps = pswt.tile([P, 2 * KD], F32)
            for k in range(2 * KD):
                nc.tensor.matmul(
                    wcol_ps[:, k : k + 1],
                    lhsT=w_row[:, k * P : (k + 1) * P],
                    rhs=one1,
                    start=True,
                    stop=True,
                )
            wcol = singles.tile([P, 2 * KD + 2], F16)
            nc.vector.memset(wcol[:, KD : KD + 1], 0.0)
            nc.vector.memset(wcol[:, 2 * KD + 1 : 2 * KD + 2], 0.0)
            nc.vector.tensor_copy(wcol[:, 0:KD], wcol_ps[:, 0:KD])
            nc.vector.tensor_copy(wcol[:, KD + 1 : 2 * KD + 1], wcol_ps[:, KD:])

            # per-partition (scale, bias) selector vectors over 2
            # partitions, built from iota so no partition-offset writes:
            # i01 = (0, 1); i10 = (1, 0); bi_r = (1, b).
            i01 = singles.tile([2, 1], F32)
            nc.gpsimd.iota(
                i01, [[0, 1]], channel_multiplier=1,
                allow_small_or_imprecise_dtypes=True,
            )
            i10 = singles.tile([2, 1], F32)
            nc.vector.tensor_scalar(
                out=i10, in0=i01, scalar1=-1.0, scalar2=1.0,
                op0=mybir.AluOpType.mult, op1=mybir.AluOpType.add,
            )
            bi_r = singles.tile([2, 1], F32)
            nc.vector.tensor_scalar(
                out=bi_r, in0=i01, scalar1=b_const - 1.0, scalar2=1.0,
                op0=mybir.AluOpType.mult, op1=mybir.AluOpType.add,
            )

            # ---- all transposed loads up front on the sync ring ----
            dep_tiles = []
            head_tiles = []
            for bi in range(BPC):
                dep_t = []
                head_t = []
                for k in range(KD):
                    dt_ = loads.tile([P, S], F16, tag="dep")
                    nc.sync.dma_start_transpose(
                        out=dt_, in_=dep[bi, :, k * P : (k + 1) * P]
                    )
                    dep_t.append(dt_)
                for k in range(KD):
                    ht = loads.tile([P, S], F16, tag="head")
                    nc.sync.dma_start_transpose(
                        out=ht, in_=head[bi, :, k * P : (k + 1) * P]
                    )
                    head_t.append(ht)
                dep_tiles.append(dep_t)
                head_tiles.append(head_t)

            for bi in range(BPC):
                # ---- dot products: 12 accumulating matmuls into
                # rp = (sh; sd) [2, S]. head k lhsT picks cols (k, 13)
                # = (w_h_k, 0); dep k picks (6, 7+k) = (0, w_d_k), so
                # each side contributes zero to the other row. ----
                rp = psrows.tile([2, S], F32, tag="rp")
                for k in range(KD):
                    for j0 in (0, 512):  # psum bank boundary at 512 f32
                        nc.tensor.matmul(
                            rp[:, j0 : j0 + 512],
                            lhsT=wcol[:, KD : KD + 2 + k : 1 + k],
                            rhs=dep_tiles[bi][k][:, j0 : j0 + 512],
                            start=(k == 0),
                            stop=False,
                        )
                for k in range(KD):
                    for j0 in (0, 512):
                        nc.tensor.matmul(
                            rp[:, j0 : j0 + 512],
                            lhsT=wcol[:, k : 2 * KD + 2 : 2 * KD + 1 - k],
                            rhs=head_tiles[bi][k][:, j0 : j0 + 512],
                            start=False,
                            stop=(k == KD - 1),
                        )
                # rowl = (sh, 1) and rowr = (1, sd+b) in fp16 SBUF, each
                # one ACT op with per-partition scale/bias selectors.
                rowl = outs.tile([2, S], F16, tag="rowl", bufs=2)
                nc.scalar.activation(
                    out=rowl, in_=rp,
                    func=mybir.ActivationFunctionType.Identity,
                    bias=i01, scale=i10,
                )
                rowr = outs.tile([2, S], F16, tag="rowr", bufs=2)
                nc.scalar.activation(
                    out=rowr, in_=rp,
                    func=mybir.ActivationFunctionType.Identity,
                    bias=bi_r, scale=i01,
                )

                # ---- output grid: one K=2 matmul per 128-row chunk ----
                for c in range(C):
                    op = psout.tile([P, S], F32, tag="op")
                    for j0 in (0, 512):
                        nc.tensor.matmul(
                            op[:, j0 : j0 + 512],
                            lhsT=rowl[:, c * P : (c + 1) * P],
                            rhs=rowr[:, j0 : j0 + 512],
                            start=True,
                            stop=True,
                        )
                    ot = outs.tile([P, S], F16, tag="ot")
                    if EVAC_ENG[c] == "V":
                        nc.vector.tensor_copy(ot, op)
                        nc.sync.dma_start(out=out_v[bi, c], in_=ot)
                    else:
                        nc.scalar.copy(out=ot, in_=op)
                        nc.scalar.dma_start(out=out_v[bi, c], in_=ot)
    nc.compile()
    return nc


def kernel(head, dep, edge_W, edge_b, _trace=False):
    nc = build_program(float(edge_b[0]))
    head16 = head.astype(np.float16)
    dep16 = dep.astype(np.float16)
    w16 = edge_W.astype(np.float16)
    in_maps = []
    for k in range(N_CORES):
        in_maps.append(
            {
                "head": np.ascontiguousarray(head16[k * BPC : (k + 1) * BPC]),
                "dep": np.ascontiguousarray(dep16[k * BPC : (k + 1) * BPC]),
                "edge_W": w16,
            }
        )
    res = run_bass_kernel_spmd(nc, in_maps, core_ids=list(range(N_CORES)), trace=_trace)
    out = np.concatenate([r["out"] for r in res.results], axis=0).astype(np.float32)
    if _trace:
        return out, res
    return out


if __name__ == "__main__":
    rng = np.random.default_rng(0)
    head = rng.standard_normal((B, S, D), dtype=np.float32)
    dep = rng.standard_normal((B, S, D), dtype=np.float32)
    edge_W = rng.standard_normal((1, 2 * D), dtype=np.float32)
    edge_b = rng.standard_normal((1,), dtype=np.float32)
    out = kernel(head, dep, edge_W, edge_b)
    ref = (
        head @ edge_W[0, :D]
    )[:, :, None] + (dep @ edge_W[0, D:])[:, None, :] + edge_b[0]
    err = np.abs(out - ref).max() / np.abs(ref).max()
    print("max rel err:", err)


# revision 22
# speedup vs baseline: 1.0926x; 1.0926x over previous
"""AffineEdgeAttention Trainium2 kernel (fp16 I/O, no-transpose).

out[b, i, j] = head[b, i] . w_h + dep[b, j] . w_d + edge_b
with w_h = edge_W[0, :D], w_d = edge_W[0, D:].

Sharding: data-parallel over batch; 16 batches / 8 cores = 2 per core.

All HBM traffic is fp16 (gate is 2e-2; fp16 keeps ~5e-4): per core
3+3 MiB loads + 4 MiB stores ~= 10.25 MiB -> ~29 us floor at the
358 GB/s per-NC HBM limit. Loads stay contiguous (DMA-transpose runs
at ~186 GB/s serialized on its HWDGE engine and corrupts when two
transpose streams overlap, so it is avoided entirely). Work is spread
so every engine stays under ~20 us and the kernel is DMA-paced:

  dep path (GpSimd+ACT+PE): per pair tile one GpSimd tensor_mul
    (*w_d broadcast), two ACT accum-reduces -> sd columns; each column
    is transposed+broadcast into PSUM [128, S] by a K=128 matmul with
    a stride-0 stationary and IDENTITY rhs (the one stride-0-stationary
    form that is correct on HW); one ACT op folds +edge_b -> fp16 sdb.
  head path (DVE): one fused scalar_tensor_tensor per 128-row chunk
    (multiply + free-axis accumulate, 875 ns) -> sh columns (f32).
  output adds: tensor_scalar_add on DVE (4x fp16 mode, ~0.5 us) for
    most pairs, ACT activation-add / GpSimd tensor_add for the rest;
    stores split across the sync/scalar HWDGE rings.
edge_b is baked in at trace time via memset (known host-side).
"""

import sys

import numpy as np

for _p in ("/opt/trn_rl_repo", "/root/.axon_site/_ro/trn_rl_repo"):
    if _p not in sys.path:
        sys.path.insert(0, _p)

import concourse.bacc as bacc
import concourse.bass as bass
import concourse.tile as tile
from concourse import mybir
from concourse.bass_utils import run_bass_kernel_spmd

B, S, D = 16, 1024, 768
N_CORES = 8
BPC = B // N_CORES  # batches per core
P = 128
C = S // P  # 8 row-chunks of 128
NPAIR = C // 2  # 4 chunk-pair tiles per tensor per batch

F16 = mybir.dt.float16
F32 = mybir.dt.float32

# per-pair output engine for adds+store ring: V pairs store on sync,
# A/G pairs on scalar. Per batch: [V, V, A, G] keeps V ~19us, A ~19us,
# G ~18us overall.
OUT_PAIR_ENG = ["V", "V", "A", "G"]


def build_program(b_const: float) -> bass.Bass:
    nc = bacc.Bacc("TRN2", target_bir_lowering=False, debug=False)
    head = nc.dram_tensor("head", [BPC, S, D], F16, kind="ExternalInput").ap()
    dep = nc.dram_tensor("dep", [BPC, S, D], F16, kind="ExternalInput").ap()
    w = nc.dram_tensor("edge_W", [1, 2 * D], F16, kind="ExternalInput").ap()
    out = nc.dram_tensor("out", [BPC, S, S], F16, kind="ExternalOutput").ap()

    # [b, t, p, c, d]: chunk-pair t, intra-pair c; rows (2t+c)*128+p
    head_v = head.rearrange("b (t c p) d -> b t p c d", c=2, p=P)
    dep_v = dep.rearrange("b (t c p) d -> b t p c d", c=2, p=P)
    out_v = out.rearrange("b (t c p) j -> b t p c j", c=2, p=P)

    with tile.TileContext(nc) as tc:
        with (
            tc.tile_pool(name="singles", bufs=1) as singles,
            tc.tile_pool(name="loads", bufs=2 * NPAIR) as loads,
            tc.tile_pool(name="svec", bufs=2) as svec,
            tc.tile_pool(name="scratch", bufs=2) as scratch,
            tc.tile_pool(name="bcast", bufs=2) as bcast,
            tc.tile_pool(name="outs", bufs=6) as outs,
            tc.tile_pool(name="psd", bufs=2, space="PSUM") as psd,
            tc.tile_pool(name="psinit", bufs=1, space="PSUM") as psinit,
        ):
            # ---- constants ----
            iota_f = singles.tile([P, P], F32)
            nc.gpsimd.iota(
                iota_f, [[1, P]], channel_multiplier=0,
                allow_small_or_imprecise_dtypes=True,
            )
            iota_p = singles.tile([P, 1], F32)
            nc.gpsimd.iota(
                iota_p, [[0, 1]], channel_multiplier=1,
                allow_small_or_imprecise_dtypes=True,
            )
            ident = singles.tile([P, P], F32)
            nc.vector.tensor_scalar(
                out=ident, in0=iota_f, scalar1=iota_p, scalar2=None,
                op0=mybir.AluOpType.is_equal,
            )
            w_row = singles.tile([1, 2 * D], F16)
            nc.scalar.dma_start(out=w_row, in_=w)
            ones = singles.tile([1, P], F16)
            nc.vector.memset(ones, 1.0)
            bt = singles.tile([P, 1], F32)
            nc.vector.memset(bt, b_const)

            # w_h / w_d broadcast to all 128 partitions, fp16 SBUF
            psw_d = psinit.tile([P, D], F32)
            psw_h = psinit.tile([P, D], F32)
            for dst, lo in ((psw_d, D), (psw_h, 0)):
                for k0, k1 in ((0, 512), (512, D)):  # psum bank boundary
                    nc.tensor.matmul(
                        dst[:, k0:k1],
                        lhsT=ones,
                        rhs=w_row[:, lo + k0 : lo + k1],
                        start=True,
                        stop=True,
                    )
            wtd = singles.tile([P, D], F16)
            nc.scalar.copy(out=wtd, in_=psw_d)
            wth = singles.tile([P, D], F16)
            nc.scalar.copy(out=wth, in_=psw_h)

            # ---- all loads up front on the sync ring; dep first per
            # batch (its chain to sdb is longer) ----
            dep_tiles = []
            head_tiles = []
            for bi in range(BPC):
                dep_t = []
                for t in range(NPAIR):
                    dt_ = loads.tile([P, 2, D], F16, tag="dep")
                    nc.sync.dma_start(out=dt_, in_=dep_v[bi, t])
                    dep_t.append(dt_)
                head_t = []
                for t in range(NPAIR):
                    ht = loads.tile([P, 2, D], F16, tag="head")
                    nc.sync.dma_start(out=ht, in_=head_v[bi, t])
                    head_t.append(ht)
                dep_tiles.append(dep_t)
                head_tiles.append(head_t)

            for bi in range(BPC):
                # ---- s_d: GpSimd pair multiply, ACT accum-reduce per
                # chunk, PE transpose+broadcast into PSUM ----
                sd = svec.tile([P, C], F32, tag="sd")
                ps = psd.tile([P, S], F32, tag="ps")
                for t in range(NPAIR):
                    src = dep_tiles[bi][t]
                    prod = scratch.tile([P, 2, D], F16, tag="prodG")
                    nc.gpsimd.tensor_mul(
                        prod,
                        src,
                        wtd.rearrange("p (o d) -> p o d", o=1).broadcast_to(
                            (P, 2, D)
                        ),
                    )
                    for i in range(2):
                        c = 2 * t + i
                        nc.scalar.activation(
                            out=prod[:, i, :],
                            in_=prod[:, i, :],
                            func=mybir.ActivationFunctionType.Copy,
                            accum_out=sd[:, c : c + 1],
                        )
                    for k in (2 * t, 2 * t + 1):
                        nc.tensor.matmul(
                            ps[:, k * P : (k + 1) * P],
                            lhsT=sd[:, k : k + 1].broadcast_to((P, P)),
                            rhs=ident,
                            start=True,
                            stop=True,
                        )
                sdb = bcast.tile([P, S], F16, tag="sdb")
                nc.scalar.add(out=sdb, in_=ps, add=bt)

                # ---- s_h chunks (DVE fused) + output chunks ----
                sh = svec.tile([P, C], F32, tag="sh")
                for t in range(NPAIR):
                    for i in range(2):
                        c = 2 * t + i
                        prod = scratch.tile([P, D], F16, tag="prodV")
                        nc.vector.scalar_tensor_tensor(
                            out=prod,
                            in0=head_tiles[bi][t][:, i, :],
                            scalar=0.0,
                            in1=wth,
                            op0=mybir.AluOpType.bypass,
                            op1=mybir.AluOpType.mult,
                            accum_out=sh[:, c : c + 1],
                        )
                    ot = outs.tile([P, 2, S], F16, tag="ot")
                    eng = OUT_PAIR_ENG[t]
                    for i in range(2):
                        c = 2 * t + i
                        if eng == "V":
                            nc.vector.tensor_scalar_add(
                                ot[:, i, :], sdb, sh[:, c : c + 1]
                            )
                        elif eng == "A":
                            nc.scalar.add(
                                out=ot[:, i, :], in_=sdb, add=sh[:, c : c + 1]
                            )
                        else:
                            nc.gpsimd.tensor_add(
                                ot[:, i, :],
                                sdb,
                                sh[:, c : c + 1].broadcast_to((P, S)),
                            )
                    if eng == "V":
                        nc.sync.dma_start(out=out_v[bi, t], in_=ot)
                    else:
                        nc.scalar.dma_start(out=out_v[bi, t], in_=ot)
    nc.compile()
    return nc


def kernel(head, dep, edge_W, edge_b, _trace=False):
    nc = build_program(float(edge_b[0]))
    head16 = head.astype(np.float16)
    dep16 = dep.astype(np.float16)
    w16 = edge_W.astype(np.float16)
    in_maps = []
    for k in range(N_CORES):
        in_maps.append(
            {
                "head": np.ascontiguousarray(head16[k * BPC : (k + 1) * BPC]),
                "dep": np.ascontiguousarray(dep16[k * BPC : (k + 1) * BPC]),
                "edge_W": w16,
            }
        )
    res = run_bass_kernel_spmd(nc, in_maps, core_ids=list(range(N_CORES)), trace=_trace)
    out = np.concatenate([r["out"] for r in res.results], axis=0).astype(np.float32)
    if _trace:
        return out, res
    return out


if __name__ == "__main__":
    rng = np.random.default_rng(0)
    head = rng.standard_normal((B, S, D), dtype=np.float32)
    dep = rng.standard_normal((B, S, D), dtype=np.float32)
    edge_W = rng.standard_normal((1, 2 * D), dtype=np.float32)
    edge_b = rng.standard_normal((1,), dtype=np.float32)
    out = kernel(head, dep, edge_W, edge_b)
    ref = (
        head @ edge_W[0, :D]
    )[:, :, None] + (dep @ edge_W[0, D:])[:, None, :] + edge_b[0]
    err = np.abs(out - ref).max() / np.abs(ref).max()
    print("max rel err:", err)


# revision 23
# speedup vs baseline: 1.5318x; 1.4019x over previous
"""AffineEdgeAttention Trainium2 kernel (fp16 I/O, DVE-dots).

out[b, i, j] = head[b, i] . w_h + dep[b, j] . w_d + edge_b
with w_h = edge_W[0, :D], w_d = edge_W[0, D:].

Sharding: data-parallel over batch; 16 batches / 8 cores = 2 per core.

All HBM traffic is fp16 (gate is 2e-2; fp16 keeps ~5e-4): per core
3+3 MiB loads + 4 MiB stores ~= 10.25 MiB -> ~29 us floor at the
358 GB/s per-NC HBM limit. Loads stay contiguous: the xbar DMA
transpose runs at ~186 GB/s serialized on its issuing engine and two
concurrent transpose streams corrupt each other, so it is avoided.
GpSimd is left idle: its f16 tensor ops measure ~3.1 us per pair tile
AND slow concurrent DVE ops ~4x (shared SBUF path), so using it is
strictly worse than leaving everything on DVE.

  dots (DVE): one fused scalar_tensor_tensor per 128-row chunk
    (multiply by the w broadcast + free-axis accumulate, 870 ns at 1x)
    -> sd / sh columns (f32). 32 chunks ~= 28 us, streaming right
    behind the loads; this co-paces with the DMA floor.
  s_d broadcast: per column one K=128 matmul with stride-0 stationary
    and IDENTITY rhs transposes+broadcasts into PSUM [128, S]; one ACT
    op folds +edge_b and casts to fp16 SBUF.
  output adds: ACT activation-add for pairs 0-2 (~1.0 us each), DVE
    tensor_scalar_add (4x fp16, ~0.4 us) for the last pair of each
    batch so the tail after the final dot is short. V-pair stores ride
    the sync ring, A-pair stores the scalar ring.
edge_b is baked in at trace time via memset (known host-side).
"""

import sys

import numpy as np

for _p in ("/opt/trn_rl_repo", "/root/.axon_site/_ro/trn_rl_repo"):
    if _p not in sys.path:
        sys.path.insert(0, _p)

import concourse.bacc as bacc
import concourse.bass as bass
import concourse.tile as tile
from concourse import mybir
from concourse.bass_utils import run_bass_kernel_spmd

B, S, D = 16, 1024, 768
N_CORES = 8
BPC = B // N_CORES  # batches per core
P = 128
C = S // P  # 8 row-chunks of 128
NPAIR = C // 2  # 4 chunk-pair tiles per tensor per batch

F16 = mybir.dt.float16
F32 = mybir.dt.float32

# per-pair output engine: V takes the last pair of each batch (short
# tail right after its own dots), ACT the rest.
OUT_PAIR_ENG = ["A", "A", "A", "V"]


def build_program(b_const: float) -> bass.Bass:
    nc = bacc.Bacc("TRN2", target_bir_lowering=False, debug=False)
    head = nc.dram_tensor("head", [BPC, S, D], F16, kind="ExternalInput").ap()
    dep = nc.dram_tensor("dep", [BPC, S, D], F16, kind="ExternalInput").ap()
    w = nc.dram_tensor("edge_W", [1, 2 * D], F16, kind="ExternalInput").ap()
    out = nc.dram_tensor("out", [BPC, S, S], F16, kind="ExternalOutput").ap()

    # [b, t, p, c, d]: chunk-pair t, intra-pair c; rows (2t+c)*128+p
    head_v = head.rearrange("b (t c p) d -> b t p c d", c=2, p=P)
    dep_v = dep.rearrange("b (t c p) d -> b t p c d", c=2, p=P)
    out_v = out.rearrange("b (t c p) j -> b t p c j", c=2, p=P)

    with tile.TileContext(nc) as tc:
        with (
            tc.tile_pool(name="singles", bufs=1) as singles,
            tc.tile_pool(name="loads", bufs=2 * NPAIR) as loads,
            tc.tile_pool(name="svec", bufs=2) as svec,
            tc.tile_pool(name="scratch", bufs=2) as scratch,
            tc.tile_pool(name="bcast", bufs=2) as bcast,
            tc.tile_pool(name="outs", bufs=6) as outs,
            tc.tile_pool(name="psd", bufs=2, space="PSUM") as psd,
            tc.tile_pool(name="psinit", bufs=1, space="PSUM") as psinit,
        ):
            # ---- constants ----
            iota_f = singles.tile([P, P], F32)
            nc.gpsimd.iota(
                iota_f, [[1, P]], channel_multiplier=0,
                allow_small_or_imprecise_dtypes=True,
            )
            iota_p = singles.tile([P, 1], F32)
            nc.gpsimd.iota(
                iota_p, [[0, 1]], channel_multiplier=1,
                allow_small_or_imprecise_dtypes=True,
            )
            ident = singles.tile([P, P], F32)
            nc.vector.tensor_scalar(
                out=ident, in0=iota_f, scalar1=iota_p, scalar2=None,
                op0=mybir.AluOpType.is_equal,
            )
            w_row = singles.tile([1, 2 * D], F16)
            nc.scalar.dma_start(out=w_row, in_=w)
            ones = singles.tile([1, P], F16)
            nc.vector.memset(ones, 1.0)
            bt = singles.tile([P, 1], F32)
            nc.vector.memset(bt, b_const)

            # w_h / w_d broadcast to all 128 partitions, fp16 SBUF
            psw_d = psinit.tile([P, D], F32)
            psw_h = psinit.tile([P, D], F32)
            for dst, lo in ((psw_d, D), (psw_h, 0)):
                for k0, k1 in ((0, 512), (512, D)):  # psum bank boundary
                    nc.tensor.matmul(
                        dst[:, k0:k1],
                        lhsT=ones,
                        rhs=w_row[:, lo + k0 : lo + k1],
                        start=True,
                        stop=True,
                    )
            wtd = singles.tile([P, D], F16)
            nc.scalar.copy(out=wtd, in_=psw_d)
            wth = singles.tile([P, D], F16)
            nc.scalar.copy(out=wth, in_=psw_h)

            # ---- all loads up front on the sync ring; dep first per
            # batch (its chain to sdb is longer) ----
            dep_tiles = []
            head_tiles = []
            for bi in range(BPC):
                dep_t = []
                for t in range(NPAIR):
                    dt_ = loads.tile([P, 2, D], F16, tag="dep")
                    nc.sync.dma_start(out=dt_, in_=dep_v[bi, t])
                    dep_t.append(dt_)
                head_t = []
                for t in range(NPAIR):
                    ht = loads.tile([P, 2, D], F16, tag="head")
                    nc.sync.dma_start(out=ht, in_=head_v[bi, t])
                    head_t.append(ht)
                dep_tiles.append(dep_t)
                head_tiles.append(head_t)

            def fused_dot(src_chunk, w_tile, acc_col):
                """acc_col[p] = sum_d src_chunk[p, d] * w_tile[p, d] (DVE)."""
                prod = scratch.tile([P, D], F16, tag="prodV", name="prod")
                nc.vector.scalar_tensor_tensor(
                    out=prod,
                    in0=src_chunk,
                    scalar=0.0,
                    in1=w_tile,
                    op0=mybir.AluOpType.bypass,
                    op1=mybir.AluOpType.mult,
                    accum_out=acc_col,
                )

            for bi in range(BPC):
                # ---- s_d columns (DVE) -> PE transpose+broadcast ----
                sd = svec.tile([P, C], F32, tag="sd")
                ps = psd.tile([P, S], F32, tag="ps")
                for t in range(NPAIR):
                    for i in range(2):
                        c = 2 * t + i
                        fused_dot(
                            dep_tiles[bi][t][:, i, :], wtd, sd[:, c : c + 1]
                        )
                    for k in (2 * t, 2 * t + 1):
                        nc.tensor.matmul(
                            ps[:, k * P : (k + 1) * P],
                            lhsT=sd[:, k : k + 1].broadcast_to((P, P)),
                            rhs=ident,
                            start=True,
                            stop=True,
                        )
                sdb = bcast.tile([P, S], F16, tag="sdb")
                nc.scalar.add(out=sdb, in_=ps, add=bt)

                # ---- s_h chunks + output chunks ----
                sh = svec.tile([P, C], F32, tag="sh")
                for t in range(NPAIR):
                    for i in range(2):
                        c = 2 * t + i
                        fused_dot(
                            head_tiles[bi][t][:, i, :], wth, sh[:, c : c + 1]
                        )
                    ot = outs.tile([P, 2, S], F16, tag="ot")
                    for i in range(2):
                        c = 2 * t + i
                        if OUT_PAIR_ENG[t] == "A":
                            nc.scalar.add(
                                out=ot[:, i, :], in_=sdb, add=sh[:, c : c + 1]
                            )
                        else:
                            nc.vector.tensor_scalar_add(
                                ot[:, i, :], sdb, sh[:, c : c + 1]
                            )
                    if OUT_PAIR_ENG[t] == "A":
                        nc.scalar.dma_start(out=out_v[bi, t], in_=ot)
                    else:
                        nc.sync.dma_start(out=out_v[bi, t], in_=ot)
    nc.compile()
    return nc


def kernel(head, dep, edge_W, edge_b, _trace=False):
    nc = build_program(float(edge_b[0]))
    head16 = head.astype(np.float16)
    dep16 = dep.astype(np.float16)
    w16 = edge_W.astype(np.float16)
    in_maps = []
    for k in range(N_CORES):
        in_maps.append(
            {
                "head": np.ascontiguousarray(head16[k * BPC : (k + 1) * BPC]),
                "dep": np.ascontiguousarray(dep16[k * BPC : (k + 1) * BPC]),
                "edge_W": w16,
            }
        )
    res = run_bass_kernel_spmd(nc, in_maps, core_ids=list(range(N_CORES)), trace=_trace)
    out = np.concatenate([r["out"] for r in res.results], axis=0).astype(np.float32)
    if _trace:
        return out, res
    return out


if __name__ == "__main__":
    rng = np.random.default_rng(0)
    head = rng.standard_normal((B, S, D), dtype=np.float32)
    dep = rng.standard_normal((B, S, D), dtype=np.float32)
    edge_W = rng.standard_normal((1, 2 * D), dtype=np.float32)
    edge_b = rng.standard_normal((1,), dtype=np.float32)
    out = kernel(head, dep, edge_W, edge_b)
    ref = (
        head @ edge_W[0, :D]
    )[:, :, None] + (dep @ edge_W[0, D:])[:, None, :] + edge_b[0]
    err = np.abs(out - ref).max() / np.abs(ref).max()
    print("max rel err:", err)
